# revision 12
# baseline (speedup 1.0000x reference)
"""KnowledgeAwareAttention TRN2 kernel — flat masked-sum architecture.

attn[i,j] = sum_d R_emb[q[i,j],d] * x[j,d] * x[i,d]
out = softmax(attn, -1) @ x

Per core (128 rows):
  attn = sum_{k=1..41} 1[q==k] * T_k   (T_0 == 0: R row 0 is zeroed)
  - PE: T_k = (x_I*R_k*512) @ x^T via fp8e4 DoubleRow matmuls (256-contraction
    per instruction, ldweights pipelined away).
  - Planes are evacuated PSUM->SBUF as fp8 into pair-tiles (ScalarE, with a
    VectorE share for balance), then masked IN PLACE by a single
    tensor_tensor(bitwise_and) against host-packed one-hot lane masks
    (0xFF per selected fp8 lane, uint16-packed) - ~2x cheaper than any
    predicated/stt op on DVE.
  - PE accumulates masked pairs into the attn PSUM tile via dual-identity
    DoubleRow matmuls (two planes per 512-col call).
  - exp on ScalarE with scale=1/512 (undoes the fp8 range scale) + fused
    row-sum; reciprocal on VectorE; transposes + output matmul in bf16.
  - lh prep (x_I * R * 512 in fp8 DR layout) in ONE VectorE broadcast op.
"""

import numpy as np
import ml_dtypes

import concourse.bass as bass
import concourse.mybir as mybir
import concourse.tile as tile
from concourse.bass_utils import run_bass_kernel_spmd
from concourse.masks import make_identity

B = 1024
D = 256
NREL = 42
NK = NREL - 1  # planes 1..41; plane 0 is identically zero
NCORES = 8
P = 128
F32 = mybir.dt.float32
BF16 = mybir.dt.bfloat16
FP8 = mybir.dt.float8e4
AF = mybir.ActivationFunctionType
DRM = mybir.MatmulPerfMode.DoubleRow
RSCALE = 512.0  # fp8 range scale folded into R; undone in exp

# planes evacuated by VectorE instead of ScalarE (engine balance)
DVE_EVAC = set(range(1, 42, 4))  # 11 planes
# pair-adds lag the plane pipeline by this many pairs so the PE never
# stalls waiting for the evac+mask chain
ADD_LAG = 4


def _patch_tile_tail_drain():
    """This container's walrus rejects >1 sync-wait command on the
    kernel-tail SP Drain. Split the waits across SP nops."""
    import concourse.mybir as mybir_
    import concourse.tile as tile_

    def _drain_and_barrier(self, tick_clock, wait_clock):
        nc = self.nc
        drain_inst = nc.sync.drain()
        wait_clock.add_sem_waits(
            drain_inst.ins, tile_.ScopedClock({None: tick_clock.global_clock})
        )
        si = drain_inst.ins.sync_info
        waits = list(si.on_wait) if si and si.on_wait else []
        if len(waits) > 1:
            si.on_wait = waits[:1]
            for w in waits[1:]:
                nop = nc.sync.nop(nofuse=True)
                nop.ins.sync_info = mybir_.SyncInfo(on_wait=[w], on_update=[])
        nc.all_engine_barrier()
        assert self.sems is not None
        popped = nc._tile_sem_poison_stack.pop()
        assert popped is self._sem_poison
        nc.clear_and_free_semaphores(list(self.sems.allocated().values()))
        nc.all_engine_barrier()

    tile_.TileContext._drain_and_barrier = _drain_and_barrier


_patch_tile_tail_drain()


_MAX_WAITS = 1


def _split_excess_waits(nc: bass.Bass, max_waits: int = _MAX_WAITS) -> None:
    """This container's walrus caps the number of sync-wait commands one
    instruction may carry. Move excess waits onto same-engine NoOps."""
    cnt = 0
    for wrapper in nc.bb_map.values():
        bb = wrapper.bb
        old = list(bb.instructions)
        new = []
        changed = False
        for ins in old:
            si = ins.sync_info
            waits = list(si.on_wait) if si and si.on_wait else []
            if len(waits) > max_waits:
                changed = True
                si.on_wait = waits[:max_waits]
                rest = waits[max_waits:]
                for i in range(0, len(rest), max_waits):
                    nop = mybir.InstNoOp(name=f"waitnop{cnt}", ins=[], outs=[])
                    cnt += 1
                    nop.engine = ins.engine
                    nop.sync_info = mybir.SyncInfo(
                        on_wait=rest[i:i + max_waits], on_update=[]
                    )
                    new.append(nop)
            new.append(ins)
        if changed:
            bb.instructions = new


def build_nc() -> bass.Bass:
    nc = bass.Bass()
    xt_d = nc.dram_tensor("xt", [P, 2 * B], FP8, kind="ExternalInput")
    xti_d = nc.dram_tensor("xti", [P, 2 * P], BF16, kind="ExternalInput")
    rt_d = nc.dram_tensor("rt", [P, 2 * NK], BF16, kind="ExternalInput")
    mku_d = nc.dram_tensor("mku", [P, (NK + 1) * 512], mybir.dt.uint16,
                           kind="ExternalInput")
    di_d = nc.dram_tensor("di", [P, 2 * P], FP8, kind="ExternalInput")
    xc_d = nc.dram_tensor("xc", [P, 8 * D], BF16, kind="ExternalInput")
    out_d = nc.dram_tensor("out", [P, D], F32, kind="ExternalOutput")

    with tile.TileContext(nc) as tc:
        with (
            tc.tile_pool(name="const", bufs=1) as const,
            tc.tile_pool(name="lh", bufs=1) as lhp,
            tc.tile_pool(name="pb", bufs=3) as pbp,
            tc.tile_pool(name="mk", bufs=5) as mkp,
            tc.tile_pool(name="sm", bufs=1) as smp,
            tc.tile_pool(name="et", bufs=4) as etp,
        ):
            # ---- loads ----
            xt_t = const.tile([P, 2 * B], FP8, tag="xt", name="xt_t")
            xti_t = const.tile([P, 2 * P], BF16, tag="xti", name="xti_t")
            rt_t = const.tile([P, 2 * NK], BF16, tag="rt", name="rt_t")
            mku_t = const.tile([P, (NK + 1) * 512], mybir.dt.uint16, tag="mku",
                               name="mku_t")
            di_t = const.tile([P, 2 * P], FP8, tag="di", name="di_t")
            xc_t = const.tile([P, 8 * D], BF16, tag="xc", name="xc_t")
            nc.sync.dma_start(xti_t[:, :], xti_d[:, :])
            nc.sync.dma_start(rt_t[:, :], rt_d[:, :])
            nc.sync.dma_start(xt_t[:, :], xt_d[:, :])
            nc.sync.dma_start(di_t[:, :], di_d[:, :])
            # masks split into 3 chunks so early planes' masks arrive first
            nc.sync.dma_start(mku_t[:, :8 * 512], mku_d[:, :8 * 512])
            nc.sync.dma_start(mku_t[:, 8 * 512:24 * 512],
                              mku_d[:, 8 * 512:24 * 512])
            nc.sync.dma_start(mku_t[:, 24 * 512:], mku_d[:, 24 * 512:])
            nc.sync.dma_start(xc_t[:, :], xc_d[:, :])
            xc = [xc_t[:, j * D:(j + 1) * D] for j in range(8)]
            ident = const.tile([P, P], BF16, tag="ident")
            make_identity(nc, ident[:, :])

            # ---- prep: lh[p, kk, i, m] = xti[p, i*128+m] * rt[p, i*41+kk]
            # (kk = k-1; fp8 out, DoubleRow block layout [2, 128] per plane)
            lh_t = lhp.tile([P, NK * 2 * P], FP8, tag="lh", name="lh_t")
            for k0, nk in ((0, 6), (6, NK - 6)):
                in0 = (
                    xti_t[:, :]
                    .rearrange("p (i m) -> p i m", i=2)
                    .unsqueeze(1)
                    .broadcast_to([P, nk, 2, P])
                )
                in1 = (
                    rt_t[:, :]
                    .rearrange("p (i k) -> p i k", i=2)[:, :, k0:k0 + nk]
                    .transpose([0, 2, 1])
                    .unsqueeze(3)
                    .broadcast_to([P, nk, 2, P])
                )
                outv = lh_t[
                    :, k0 * 2 * P:(k0 + nk) * 2 * P
                ].rearrange("p (k i m) -> p k i m", k=nk, i=2)
                nc.vector.tensor_tensor(outv, in0, in1, mybir.AluOpType.mult)
            lh = [
                lh_t[:, kk * 2 * P:(kk + 1) * 2 * P].rearrange(
                    "p (i m) -> p i m", i=2
                )
                for kk in range(NK)
            ]
            xt_dr = xt_t[:, :].rearrange("p (i j) -> p i j", i=2)

            # ---- planes + masked accumulation ----
            di_dr = di_t[:, :].rearrange("p (i m) -> p i m", i=2)
            NPAIRS = (NK + 1) // 2  # 21; pair 20 has a zeroed second half
            with (
                tc.tile_pool(name="pp", bufs=3, space="PSUM") as pp,
                tc.tile_pool(name="ap", bufs=1, space="PSUM") as app,
            ):
                attn_ps = app.tile([P, B], F32, tag="attn")
                ready = []  # completed masked pair tiles awaiting their add
                done_adds = 0

                def emit_add(pair_t, idx):
                    pr = pair_t[:, :].rearrange("p (i j) -> p i j", i=2)
                    for jh in range(2):
                        nc.tensor.matmul(
                            attn_ps[:, jh * 512:(jh + 1) * 512],
                            lhsT=di_dr,
                            rhs=pr[:, :, jh * 512:(jh + 1) * 512],
                            start=(idx == 0),
                            stop=(idx == NPAIRS - 1),
                            perf_mode=DRM,
                        )

                pair = None
                for k in range(1, NREL):
                    kk = k - 1
                    t, half = kk // 2, kk % 2
                    pt = pp.tile([P, B], F32, tag="plane", name=f"t{k}")
                    for jh in range(2):
                        nc.tensor.matmul(
                            pt[:, jh * 512:(jh + 1) * 512],
                            lhsT=lh[kk],
                            rhs=xt_dr[:, :, jh * 512:(jh + 1) * 512],
                            start=True,
                            stop=True,
                            perf_mode=DRM,
                        )
                    if half == 0:
                        pair = mkp.tile([P, 2 * B], FP8, tag="mk",
                                        name=f"mk{t}")
                    slot = pair[:, half * B:(half + 1) * B]
                    if k in DVE_EVAC:
                        nc.vector.tensor_copy(slot, pt[:, :])
                    else:
                        nc.scalar.copy(slot, pt[:, :])
                    if half == 1 or t == NPAIRS - 1:
                        # one pair-wide in-place AND; mask slot NK zeroes the
                        # garbage second half of the last (odd) pair
                        pair16 = pair[:, :].bitcast(mybir.dt.uint16)
                        nc.vector.tensor_tensor(
                            pair16,
                            pair16,
                            mku_t[:, (2 * t) * 512:(2 * t + 2) * 512],
                            mybir.AluOpType.bitwise_and,
                        )
                        ready.append(pair)
                    while len(ready) > ADD_LAG:
                        emit_add(ready.pop(0), done_adds)
                        done_adds += 1
                while ready:
                    emit_add(ready.pop(0), done_adds)
                    done_adds += 1

                # ---- exp (undo RSCALE) + row sums ----
                Ebf = smp.tile([P, B], BF16, tag="Ebf")
                z = smp.tile([P, 1], F32, tag="z")
                rz = smp.tile([P, 1], F32, tag="rz")
                nc.scalar.activation(
                    Ebf[:, :], attn_ps[:, :], AF.Exp,
                    scale=1.0 / RSCALE, accum_out=z[:, :],
                )
                nc.vector.reciprocal(rz[:, :], z[:, :])

            # ---- transposes + output matmul (bf16) ----
            with (
                tc.tile_pool(name="tp", bufs=2, space="PSUM") as tp,
                tc.tile_pool(name="op", bufs=1, space="PSUM") as op,
            ):
                out_ps = op.tile([P, D], F32, tag="out")
                for jc in range(8):
                    ptile = tp.tile([P, P], BF16, tag="tp", name=f"tp{jc}")
                    nc.tensor.transpose(
                        ptile[:, :], Ebf[:, jc * P:(jc + 1) * P], ident[:, :]
                    )
                    et = etp.tile([P, P], BF16, tag="et", name=f"et{jc}")
                    nc.scalar.copy(et[:, :], ptile[:, :])
                    nc.tensor.matmul(
                        out_ps[:, :],
                        lhsT=et[:, :],
                        rhs=xc[jc],
                        start=(jc == 0),
                        stop=(jc == 7),
                    )
                out_sb = smp.tile([P, D], F32, tag="osb")
                nc.scalar.activation(
                    out_sb[:, :], out_ps[:, :], AF.Copy, scale=rz[:, :]
                )
                nc.sync.dma_start(out_d[:, :], out_sb[:, :])
    _split_excess_waits(nc)
    return nc


_NC_CACHE = None


def _get_nc():
    global _NC_CACHE
    if _NC_CACHE is None:
        _NC_CACHE = build_nc()
    return _NC_CACHE


def make_in_maps(x, q, R):
    x = np.asarray(x, dtype=np.float32)
    q = np.asarray(q)
    R = np.asarray(R, dtype=np.float32)
    bf = ml_dtypes.bfloat16
    f8 = ml_dtypes.float8_e4m3

    xT = np.ascontiguousarray(x.T)                        # [D, B]
    # xt_dr[p, i*B + j] = x[j, i*128+p], fp8
    xt_p = np.ascontiguousarray(
        xT.reshape(2, P, B).transpose(1, 0, 2).reshape(P, 2 * B)).astype(f8)
    # rt[p, i*41 + kk] = R[kk+1, i*128+p] * 512
    rts = (R.T[:, 1:] * RSCALE)                           # [D, 41]
    rt_p = np.ascontiguousarray(
        rts.reshape(2, P, NK).transpose(1, 0, 2).reshape(P, 2 * NK)).astype(bf)
    # xc[p, jc*D + d] = x[jc*128+p, d], bf16
    x_p = np.ascontiguousarray(
        x.reshape(8, P, D).transpose(1, 0, 2).reshape(P, 8 * D)).astype(bf)

    q32 = q.astype(np.int32)
    # dual identity for DoubleRow pair-adds: di[p, i*128+m] = (m == p)
    eye8 = np.eye(P, dtype=np.float32).astype(f8)
    di = np.concatenate([eye8, eye8], axis=1)              # [128, 256]
    in_maps = []
    for c in range(NCORES):
        rows = slice(c * P, (c + 1) * P)
        qb = q32[rows]                                     # [128, B]
        # one-hot lane masks, fp8-lane 0xFF packed little-endian into u16
        mbytes = np.zeros((P, NK, B), dtype=np.uint8)
        for k in range(1, NREL):
            mbytes[:, k - 1, :] = np.where(qb == k, 0xFF, 0)
        mbytes = np.concatenate(
            [mbytes, np.zeros((P, 1, B), dtype=np.uint8)], axis=1)
        mku = np.ascontiguousarray(
            mbytes.reshape(P, (NK + 1) * B)).view(np.uint16)
        xti = xT[:, rows]                                  # [D, 128]
        xti_p = np.ascontiguousarray(
            xti.reshape(2, P, P).transpose(1, 0, 2).reshape(P, 2 * P)
        ).astype(bf)
        in_maps.append(
            {"xt": xt_p, "xti": xti_p, "rt": rt_p, "mku": mku, "di": di,
             "xc": x_p}
        )
    return in_maps


def kernel(x, x_mask, q, f, R_emb):
    in_maps = make_in_maps(x, q, R_emb)
    res = run_bass_kernel_spmd(_get_nc(), in_maps, core_ids=list(range(NCORES)))
    out = np.concatenate([res.results[c]["out"] for c in range(NCORES)], axis=0)
    return out


# revision 13
# speedup vs baseline: 1.0154x; 1.0154x over previous
"""KnowledgeAwareAttention TRN2 kernel — flat masked-sum architecture.

attn[i,j] = sum_d R_emb[q[i,j],d] * x[j,d] * x[i,d]
out = softmax(attn, -1) @ x

Per core (128 rows):
  attn = sum_{k=1..41} 1[q==k] * T_k   (T_0 == 0: R row 0 is zeroed)
  - PE: T_k = (x_I*R_k*512) @ x^T via fp8e4 DoubleRow matmuls (256-contraction
    per instruction, ldweights pipelined away).
  - Planes are evacuated PSUM->SBUF as fp8 into pair-tiles (ScalarE, with a
    VectorE share for balance), then masked IN PLACE by a single
    tensor_tensor(bitwise_and) against host-packed one-hot lane masks
    (0xFF per selected fp8 lane, uint16-packed) - ~2x cheaper than any
    predicated/stt op on DVE.
  - PE accumulates masked pairs into the attn PSUM tile via dual-identity
    DoubleRow matmuls (two planes per 512-col call).
  - exp on ScalarE with scale=1/512 (undoes the fp8 range scale) + fused
    row-sum; reciprocal on VectorE; transposes + output matmul in bf16.
  - lh prep (x_I * R * 512 in fp8 DR layout) in ONE VectorE broadcast op.
"""

import numpy as np
import ml_dtypes

import concourse.bass as bass
import concourse.mybir as mybir
import concourse.tile as tile
from concourse.bass_utils import run_bass_kernel_spmd
from concourse.masks import make_identity

B = 1024
D = 256
NREL = 42
NK = NREL - 1  # planes 1..41; plane 0 is identically zero
NCORES = 8
P = 128
F32 = mybir.dt.float32
BF16 = mybir.dt.bfloat16
FP8 = mybir.dt.float8e4
AF = mybir.ActivationFunctionType
DRM = mybir.MatmulPerfMode.DoubleRow
RSCALE = 512.0  # fp8 range scale folded into R; undone in exp

# planes evacuated by VectorE instead of ScalarE (engine balance)
DVE_EVAC = set(range(1, 42, 4))  # 11 planes
# pair-adds lag the plane pipeline by this many pairs so the PE never
# stalls waiting for the evac+mask chain
ADD_LAG = 3


def _patch_tile_tail_drain():
    """This container's walrus rejects >1 sync-wait command on the
    kernel-tail SP Drain. Split the waits across SP nops."""
    import concourse.mybir as mybir_
    import concourse.tile as tile_

    def _drain_and_barrier(self, tick_clock, wait_clock):
        nc = self.nc
        drain_inst = nc.sync.drain()
        wait_clock.add_sem_waits(
            drain_inst.ins, tile_.ScopedClock({None: tick_clock.global_clock})
        )
        si = drain_inst.ins.sync_info
        waits = list(si.on_wait) if si and si.on_wait else []
        if len(waits) > 1:
            si.on_wait = waits[:1]
            for w in waits[1:]:
                nop = nc.sync.nop(nofuse=True)
                nop.ins.sync_info = mybir_.SyncInfo(on_wait=[w], on_update=[])
        nc.all_engine_barrier()
        assert self.sems is not None
        popped = nc._tile_sem_poison_stack.pop()
        assert popped is self._sem_poison
        nc.clear_and_free_semaphores(list(self.sems.allocated().values()))
        nc.all_engine_barrier()

    tile_.TileContext._drain_and_barrier = _drain_and_barrier


_patch_tile_tail_drain()


_MAX_WAITS = 1


def _split_excess_waits(nc: bass.Bass, max_waits: int = _MAX_WAITS) -> None:
    """This container's walrus caps the number of sync-wait commands one
    instruction may carry. Move excess waits onto same-engine NoOps."""
    cnt = 0
    for wrapper in nc.bb_map.values():
        bb = wrapper.bb
        old = list(bb.instructions)
        new = []
        changed = False
        for ins in old:
            si = ins.sync_info
            waits = list(si.on_wait) if si and si.on_wait else []
            if len(waits) > max_waits:
                changed = True
                si.on_wait = waits[:max_waits]
                rest = waits[max_waits:]
                for i in range(0, len(rest), max_waits):
                    nop = mybir.InstNoOp(name=f"waitnop{cnt}", ins=[], outs=[])
                    cnt += 1
                    nop.engine = ins.engine
                    nop.sync_info = mybir.SyncInfo(
                        on_wait=rest[i:i + max_waits], on_update=[]
                    )
                    new.append(nop)
            new.append(ins)
        if changed:
            bb.instructions = new


def build_nc() -> bass.Bass:
    nc = bass.Bass()
    xt_d = nc.dram_tensor("xt", [P, 2 * B], FP8, kind="ExternalInput")
    xti_d = nc.dram_tensor("xti", [P, 2 * P], BF16, kind="ExternalInput")
    rt_d = nc.dram_tensor("rt", [P, 2 * NK], BF16, kind="ExternalInput")
    mku_d = nc.dram_tensor("mku", [P, (NK + 1) * 512], mybir.dt.uint16,
                           kind="ExternalInput")
    di_d = nc.dram_tensor("di", [P, 2 * P], FP8, kind="ExternalInput")
    xc_d = nc.dram_tensor("xc", [P, 8 * D], BF16, kind="ExternalInput")
    out_d = nc.dram_tensor("out", [P, D], F32, kind="ExternalOutput")

    with tile.TileContext(nc) as tc:
        with (
            tc.tile_pool(name="const", bufs=1) as const,
            tc.tile_pool(name="lh", bufs=1) as lhp,
            tc.tile_pool(name="pb", bufs=3) as pbp,
            tc.tile_pool(name="mk", bufs=5) as mkp,
            tc.tile_pool(name="sm", bufs=1) as smp,
            tc.tile_pool(name="et", bufs=4) as etp,
        ):
            # ---- loads ----
            xt_t = const.tile([P, 2 * B], FP8, tag="xt", name="xt_t")
            xti_t = const.tile([P, 2 * P], BF16, tag="xti", name="xti_t")
            rt_t = const.tile([P, 2 * NK], BF16, tag="rt", name="rt_t")
            mku_t = const.tile([P, (NK + 1) * 512], mybir.dt.uint16, tag="mku",
                               name="mku_t")
            di_t = const.tile([P, 2 * P], FP8, tag="di", name="di_t")
            xc_t = const.tile([P, 8 * D], BF16, tag="xc", name="xc_t")
            nc.sync.dma_start(xti_t[:, :], xti_d[:, :])
            nc.sync.dma_start(rt_t[:, :], rt_d[:, :])
            nc.sync.dma_start(xt_t[:, :], xt_d[:, :])
            nc.sync.dma_start(di_t[:, :], di_d[:, :])
            # masks split into 3 chunks so early planes' masks arrive first
            nc.sync.dma_start(mku_t[:, :8 * 512], mku_d[:, :8 * 512])
            nc.sync.dma_start(mku_t[:, 8 * 512:24 * 512],
                              mku_d[:, 8 * 512:24 * 512])
            nc.sync.dma_start(mku_t[:, 24 * 512:], mku_d[:, 24 * 512:])
            nc.sync.dma_start(xc_t[:, :], xc_d[:, :])
            xc = [xc_t[:, j * D:(j + 1) * D] for j in range(8)]
            ident = const.tile([P, P], BF16, tag="ident")
            make_identity(nc, ident[:, :])

            # ---- prep: lh[p, kk, i, m] = xti[p, i*128+m] * rt[p, i*41+kk]
            # (kk = k-1; fp8 out, DoubleRow block layout [2, 128] per plane)
            lh_t = lhp.tile([P, NK * 2 * P], FP8, tag="lh", name="lh_t")
            for k0, nk in ((0, 6), (6, 8), (14, NK - 14)):
                in0 = (
                    xti_t[:, :]
                    .rearrange("p (i m) -> p i m", i=2)
                    .unsqueeze(1)
                    .broadcast_to([P, nk, 2, P])
                )
                in1 = (
                    rt_t[:, :]
                    .rearrange("p (i k) -> p i k", i=2)[:, :, k0:k0 + nk]
                    .transpose([0, 2, 1])
                    .unsqueeze(3)
                    .broadcast_to([P, nk, 2, P])
                )
                outv = lh_t[
                    :, k0 * 2 * P:(k0 + nk) * 2 * P
                ].rearrange("p (k i m) -> p k i m", k=nk, i=2)
                nc.vector.tensor_tensor(outv, in0, in1, mybir.AluOpType.mult)
            lh = [
                lh_t[:, kk * 2 * P:(kk + 1) * 2 * P].rearrange(
                    "p (i m) -> p i m", i=2
                )
                for kk in range(NK)
            ]
            xt_dr = xt_t[:, :].rearrange("p (i j) -> p i j", i=2)

            # ---- planes + masked accumulation ----
            di_dr = di_t[:, :].rearrange("p (i m) -> p i m", i=2)
            NPAIRS = (NK + 1) // 2  # 21; pair 20 has a zeroed second half
            with (
                tc.tile_pool(name="pp", bufs=3, space="PSUM") as pp,
                tc.tile_pool(name="ap", bufs=1, space="PSUM") as app,
            ):
                attn_ps = app.tile([P, B], F32, tag="attn")
                ready = []  # completed masked pair tiles awaiting their add
                done_adds = 0

                def emit_add(pair_t, idx):
                    pr = pair_t[:, :].rearrange("p (i j) -> p i j", i=2)
                    for jh in range(2):
                        nc.tensor.matmul(
                            attn_ps[:, jh * 512:(jh + 1) * 512],
                            lhsT=di_dr,
                            rhs=pr[:, :, jh * 512:(jh + 1) * 512],
                            start=(idx == 0),
                            stop=(idx == NPAIRS - 1),
                            perf_mode=DRM,
                        )

                pair = None
                for k in range(1, NREL):
                    kk = k - 1
                    t, half = kk // 2, kk % 2
                    pt = pp.tile([P, B], F32, tag="plane", name=f"t{k}")
                    for jh in range(2):
                        nc.tensor.matmul(
                            pt[:, jh * 512:(jh + 1) * 512],
                            lhsT=lh[kk],
                            rhs=xt_dr[:, :, jh * 512:(jh + 1) * 512],
                            start=True,
                            stop=True,
                            perf_mode=DRM,
                        )
                    if half == 0:
                        pair = mkp.tile([P, 2 * B], FP8, tag="mk",
                                        name=f"mk{t}")
                    slot = pair[:, half * B:(half + 1) * B]
                    if k in DVE_EVAC:
                        nc.vector.tensor_copy(slot, pt[:, :])
                    else:
                        nc.scalar.copy(slot, pt[:, :])
                    if half == 1 or t == NPAIRS - 1:
                        # one pair-wide in-place AND; mask slot NK zeroes the
                        # garbage second half of the last (odd) pair
                        pair16 = pair[:, :].bitcast(mybir.dt.uint16)
                        nc.vector.tensor_tensor(
                            pair16,
                            pair16,
                            mku_t[:, (2 * t) * 512:(2 * t + 2) * 512],
                            mybir.AluOpType.bitwise_and,
                        )
                        ready.append(pair)
                    while len(ready) > ADD_LAG:
                        emit_add(ready.pop(0), done_adds)
                        done_adds += 1
                while ready:
                    emit_add(ready.pop(0), done_adds)
                    done_adds += 1

                # ---- exp (undo RSCALE) + row sums, halves so the
                # transpose pipeline starts sooner ----
                Ebf = smp.tile([P, B], BF16, tag="Ebf")
                z2 = smp.tile([P, 2], F32, tag="z2")
                z = smp.tile([P, 1], F32, tag="z")
                rz = smp.tile([P, 1], F32, tag="rz")
                for jh in range(2):
                    nc.scalar.activation(
                        Ebf[:, jh * 512:(jh + 1) * 512],
                        attn_ps[:, jh * 512:(jh + 1) * 512], AF.Exp,
                        scale=1.0 / RSCALE, accum_out=z2[:, jh:jh + 1],
                    )
                nc.vector.tensor_tensor(
                    z[:, :], z2[:, 0:1], z2[:, 1:2], mybir.AluOpType.add
                )
                nc.vector.reciprocal(rz[:, :], z[:, :])

            # ---- transposes + output matmul (bf16) ----
            with (
                tc.tile_pool(name="tp", bufs=4, space="PSUM") as tp,
                tc.tile_pool(name="op", bufs=1, space="PSUM") as op,
            ):
                out_ps = op.tile([P, D], F32, tag="out")
                for jc in range(8):
                    ptile = tp.tile([P, P], BF16, tag="tp", name=f"tp{jc}")
                    nc.tensor.transpose(
                        ptile[:, :], Ebf[:, jc * P:(jc + 1) * P], ident[:, :]
                    )
                    et = etp.tile([P, P], BF16, tag="et", name=f"et{jc}")
                    nc.scalar.copy(et[:, :], ptile[:, :])
                    nc.tensor.matmul(
                        out_ps[:, :],
                        lhsT=et[:, :],
                        rhs=xc[jc],
                        start=(jc == 0),
                        stop=(jc == 7),
                    )
                out_sb = smp.tile([P, D], F32, tag="osb")
                nc.scalar.activation(
                    out_sb[:, :], out_ps[:, :], AF.Copy, scale=rz[:, :]
                )
                nc.sync.dma_start(out_d[:, :], out_sb[:, :])
    _split_excess_waits(nc)
    return nc


_NC_CACHE = None


def _get_nc():
    global _NC_CACHE
    if _NC_CACHE is None:
        _NC_CACHE = build_nc()
    return _NC_CACHE


def make_in_maps(x, q, R):
    x = np.asarray(x, dtype=np.float32)
    q = np.asarray(q)
    R = np.asarray(R, dtype=np.float32)
    bf = ml_dtypes.bfloat16
    f8 = ml_dtypes.float8_e4m3

    xT = np.ascontiguousarray(x.T)                        # [D, B]
    # xt_dr[p, i*B + j] = x[j, i*128+p], fp8
    xt_p = np.ascontiguousarray(
        xT.reshape(2, P, B).transpose(1, 0, 2).reshape(P, 2 * B)).astype(f8)
    # rt[p, i*41 + kk] = R[kk+1, i*128+p] * 512
    rts = (R.T[:, 1:] * RSCALE)                           # [D, 41]
    rt_p = np.ascontiguousarray(
        rts.reshape(2, P, NK).transpose(1, 0, 2).reshape(P, 2 * NK)).astype(bf)
    # xc[p, jc*D + d] = x[jc*128+p, d], bf16
    x_p = np.ascontiguousarray(
        x.reshape(8, P, D).transpose(1, 0, 2).reshape(P, 8 * D)).astype(bf)

    q32 = q.astype(np.int32)
    # dual identity for DoubleRow pair-adds: di[p, i*128+m] = (m == p)
    eye8 = np.eye(P, dtype=np.float32).astype(f8)
    di = np.concatenate([eye8, eye8], axis=1)              # [128, 256]
    in_maps = []
    for c in range(NCORES):
        rows = slice(c * P, (c + 1) * P)
        qb = q32[rows]                                     # [128, B]
        # one-hot lane masks, fp8-lane 0xFF packed little-endian into u16
        mbytes = np.zeros((P, NK, B), dtype=np.uint8)
        for k in range(1, NREL):
            mbytes[:, k - 1, :] = np.where(qb == k, 0xFF, 0)
        mbytes = np.concatenate(
            [mbytes, np.zeros((P, 1, B), dtype=np.uint8)], axis=1)
        mku = np.ascontiguousarray(
            mbytes.reshape(P, (NK + 1) * B)).view(np.uint16)
        xti = xT[:, rows]                                  # [D, 128]
        xti_p = np.ascontiguousarray(
            xti.reshape(2, P, P).transpose(1, 0, 2).reshape(P, 2 * P)
        ).astype(bf)
        in_maps.append(
            {"xt": xt_p, "xti": xti_p, "rt": rt_p, "mku": mku, "di": di,
             "xc": x_p}
        )
    return in_maps


def kernel(x, x_mask, q, f, R_emb):
    in_maps = make_in_maps(x, q, R_emb)
    res = run_bass_kernel_spmd(_get_nc(), in_maps, core_ids=list(range(NCORES)))
    out = np.concatenate([res.results[c]["out"] for c in range(NCORES)], axis=0)
    return out


# revision 14
# speedup vs baseline: 1.0721x; 1.0558x over previous
"""KnowledgeAwareAttention TRN2 kernel — flat masked-sum architecture.

attn[i,j] = sum_d R_emb[q[i,j],d] * x[j,d] * x[i,d]
out = softmax(attn, -1) @ x

Per core (128 rows):
  attn = sum_{k=1..41} 1[q==k] * T_k   (T_0 == 0: R row 0 is zeroed)
  - PE: T_k = (x_I*R_k*512) @ x^T via fp8e4 DoubleRow matmuls (256-contraction
    per instruction, ldweights pipelined away).
  - Planes are evacuated PSUM->SBUF as fp8 into pair-tiles (ScalarE, with a
    VectorE share for balance), then masked IN PLACE by a single
    tensor_tensor(bitwise_and) against host-packed one-hot lane masks
    (0xFF per selected fp8 lane, uint16-packed) - ~2x cheaper than any
    predicated/stt op on DVE.
  - PE accumulates masked pairs into the attn PSUM tile via dual-identity
    DoubleRow matmuls (two planes per 512-col call).
  - exp on ScalarE with scale=1/512 (undoes the fp8 range scale) + fused
    row-sum; reciprocal on VectorE; transposes + output matmul in bf16.
  - lh prep (x_I * R * 512 in fp8 DR layout) in ONE VectorE broadcast op.
"""

import numpy as np
import ml_dtypes

import concourse.bass as bass
import concourse.mybir as mybir
import concourse.tile as tile
from concourse.bass_utils import run_bass_kernel_spmd
from concourse.masks import make_identity

B = 1024
D = 256
NREL = 42
NK = NREL - 1  # planes 1..41; plane 0 is identically zero
NCORES = 8
P = 128
F32 = mybir.dt.float32
BF16 = mybir.dt.bfloat16
FP8 = mybir.dt.float8e4
AF = mybir.ActivationFunctionType
DRM = mybir.MatmulPerfMode.DoubleRow
RSCALE = 512.0  # fp8 range scale folded into R; undone in exp

# planes evacuated by VectorE instead of ScalarE (engine balance)
DVE_EVAC = set(range(3, 42, 4))  # 10 planes
# pair-adds lag the plane pipeline by this many pairs so the PE never
# stalls waiting for the evac+mask chain
ADD_LAG = 3


def _patch_tile_tail_drain():
    """This container's walrus rejects >1 sync-wait command on the
    kernel-tail SP Drain. Split the waits across SP nops."""
    import concourse.mybir as mybir_
    import concourse.tile as tile_

    def _drain_and_barrier(self, tick_clock, wait_clock):
        nc = self.nc
        drain_inst = nc.sync.drain()
        wait_clock.add_sem_waits(
            drain_inst.ins, tile_.ScopedClock({None: tick_clock.global_clock})
        )
        si = drain_inst.ins.sync_info
        waits = list(si.on_wait) if si and si.on_wait else []
        if len(waits) > 1:
            si.on_wait = waits[:1]
            for w in waits[1:]:
                nop = nc.sync.nop(nofuse=True)
                nop.ins.sync_info = mybir_.SyncInfo(on_wait=[w], on_update=[])
        nc.all_engine_barrier()
        assert self.sems is not None
        popped = nc._tile_sem_poison_stack.pop()
        assert popped is self._sem_poison
        nc.clear_and_free_semaphores(list(self.sems.allocated().values()))

    tile_.TileContext._drain_and_barrier = _drain_and_barrier


_patch_tile_tail_drain()


_MAX_WAITS = 1


def _split_excess_waits(nc: bass.Bass, max_waits: int = _MAX_WAITS) -> None:
    """This container's walrus caps the number of sync-wait commands one
    instruction may carry. Move excess waits onto same-engine NoOps."""
    cnt = 0
    for wrapper in nc.bb_map.values():
        bb = wrapper.bb
        old = list(bb.instructions)
        new = []
        changed = False
        for ins in old:
            si = ins.sync_info
            waits = list(si.on_wait) if si and si.on_wait else []
            if len(waits) > max_waits:
                changed = True
                si.on_wait = waits[:max_waits]
                rest = waits[max_waits:]
                for i in range(0, len(rest), max_waits):
                    nop = mybir.InstNoOp(name=f"waitnop{cnt}", ins=[], outs=[])
                    cnt += 1
                    nop.engine = ins.engine
                    nop.sync_info = mybir.SyncInfo(
                        on_wait=rest[i:i + max_waits], on_update=[]
                    )
                    new.append(nop)
            new.append(ins)
        if changed:
            bb.instructions = new


def build_nc() -> bass.Bass:
    nc = bass.Bass()
    xt_d = nc.dram_tensor("xt", [P, 2 * B], FP8, kind="ExternalInput")
    xti_d = nc.dram_tensor("xti", [P, 2 * P], BF16, kind="ExternalInput")
    rt_d = nc.dram_tensor("rt", [P, 2 * NK], BF16, kind="ExternalInput")
    mku_d = nc.dram_tensor("mku", [P, (NK + 1) * 512], mybir.dt.uint16,
                           kind="ExternalInput")
    di_d = nc.dram_tensor("di", [P, 2 * P], FP8, kind="ExternalInput")
    xc_d = nc.dram_tensor("xc", [P, 8 * D], BF16, kind="ExternalInput")
    out_d = nc.dram_tensor("out", [P, D], F32, kind="ExternalOutput")

    with tile.TileContext(nc) as tc:
        with (
            tc.tile_pool(name="const", bufs=1) as const,
            tc.tile_pool(name="lh", bufs=1) as lhp,
            tc.tile_pool(name="pb", bufs=3) as pbp,
            tc.tile_pool(name="mk", bufs=5) as mkp,
            tc.tile_pool(name="sm", bufs=1) as smp,
            tc.tile_pool(name="et", bufs=4) as etp,
        ):
            # ---- loads ----
            xt_t = const.tile([P, 2 * B], FP8, tag="xt", name="xt_t")
            xti_t = const.tile([P, 2 * P], BF16, tag="xti", name="xti_t")
            rt_t = const.tile([P, 2 * NK], BF16, tag="rt", name="rt_t")
            mku_t = const.tile([P, (NK + 1) * 512], mybir.dt.uint16, tag="mku",
                               name="mku_t")
            di_t = const.tile([P, 2 * P], FP8, tag="di", name="di_t")
            xc_t = const.tile([P, 8 * D], BF16, tag="xc", name="xc_t")
            nc.sync.dma_start(xti_t[:, :], xti_d[:, :])
            nc.sync.dma_start(rt_t[:, :], rt_d[:, :])
            nc.sync.dma_start(xt_t[:, :], xt_d[:, :])
            nc.sync.dma_start(di_t[:, :], di_d[:, :])
            # masks split into 3 chunks so early planes' masks arrive first
            nc.sync.dma_start(mku_t[:, :8 * 512], mku_d[:, :8 * 512])
            nc.sync.dma_start(mku_t[:, 8 * 512:24 * 512],
                              mku_d[:, 8 * 512:24 * 512])
            nc.sync.dma_start(mku_t[:, 24 * 512:], mku_d[:, 24 * 512:])
            nc.sync.dma_start(xc_t[:, :], xc_d[:, :])
            xc = [xc_t[:, j * D:(j + 1) * D] for j in range(8)]
            ident = const.tile([P, P], BF16, tag="ident")
            make_identity(nc, ident[:, :])

            # ---- prep: lh[p, kk, i, m] = xti[p, i*128+m] * rt[p, i*41+kk]
            # (kk = k-1; fp8 out, DoubleRow block layout [2, 128] per plane)
            lh_t = lhp.tile([P, NK * 2 * P], FP8, tag="lh", name="lh_t")

            def emit_prep(k0, nk):
                in0 = (
                    xti_t[:, :]
                    .rearrange("p (i m) -> p i m", i=2)
                    .unsqueeze(1)
                    .broadcast_to([P, nk, 2, P])
                )
                in1 = (
                    rt_t[:, :]
                    .rearrange("p (i k) -> p i k", i=2)[:, :, k0:k0 + nk]
                    .transpose([0, 2, 1])
                    .unsqueeze(3)
                    .broadcast_to([P, nk, 2, P])
                )
                outv = lh_t[
                    :, k0 * 2 * P:(k0 + nk) * 2 * P
                ].rearrange("p (k i m) -> p k i m", k=nk, i=2)
                nc.vector.tensor_tensor(outv, in0, in1, mybir.AluOpType.mult)

            emit_prep(0, 6)
            emit_prep(6, 8)
            # chunks for planes 15..41 are emitted inside the plane loop so
            # VectorE's evac/mask ops are not queued behind them
            lh = [
                lh_t[:, kk * 2 * P:(kk + 1) * 2 * P].rearrange(
                    "p (i m) -> p i m", i=2
                )
                for kk in range(NK)
            ]
            xt_dr = xt_t[:, :].rearrange("p (i j) -> p i j", i=2)

            # ---- planes + masked accumulation ----
            di_dr = di_t[:, :].rearrange("p (i m) -> p i m", i=2)
            NPAIRS = (NK + 1) // 2  # 21; pair 20 has a zeroed second half
            with (
                tc.tile_pool(name="pp", bufs=3, space="PSUM") as pp,
                tc.tile_pool(name="ap", bufs=1, space="PSUM") as app,
            ):
                attn_ps = app.tile([P, B], F32, tag="attn")
                ready = []  # completed masked pair tiles awaiting their add
                done_adds = 0

                def emit_add(pair_t, idx):
                    pr = pair_t[:, :].rearrange("p (i j) -> p i j", i=2)
                    for jh in range(2):
                        nc.tensor.matmul(
                            attn_ps[:, jh * 512:(jh + 1) * 512],
                            lhsT=di_dr,
                            rhs=pr[:, :, jh * 512:(jh + 1) * 512],
                            start=(idx == 0),
                            stop=(idx == NPAIRS - 1),
                            perf_mode=DRM,
                        )

                pair = None
                for k in range(1, NREL):
                    kk = k - 1
                    t, half = kk // 2, kk % 2
                    pt = pp.tile([P, B], F32, tag="plane", name=f"t{k}")
                    for jh in range(2):
                        nc.tensor.matmul(
                            pt[:, jh * 512:(jh + 1) * 512],
                            lhsT=lh[kk],
                            rhs=xt_dr[:, :, jh * 512:(jh + 1) * 512],
                            start=True,
                            stop=True,
                            perf_mode=DRM,
                        )
                    if half == 0:
                        pair = mkp.tile([P, 2 * B], FP8, tag="mk",
                                        name=f"mk{t}")
                    slot = pair[:, half * B:(half + 1) * B]
                    if k in DVE_EVAC:
                        nc.vector.tensor_copy(slot, pt[:, :])
                    else:
                        nc.scalar.copy(slot, pt[:, :])
                    if half == 1 or t == NPAIRS - 1:
                        # one pair-wide in-place AND; mask slot NK zeroes the
                        # garbage second half of the last (odd) pair
                        pair16 = pair[:, :].bitcast(mybir.dt.uint16)
                        nc.vector.tensor_tensor(
                            pair16,
                            pair16,
                            mku_t[:, (2 * t) * 512:(2 * t + 2) * 512],
                            mybir.AluOpType.bitwise_and,
                        )
                        ready.append(pair)
                    if k == 4:
                        emit_prep(14, 14)
                    elif k == 10:
                        emit_prep(28, NK - 28)
                    while len(ready) > ADD_LAG:
                        emit_add(ready.pop(0), done_adds)
                        done_adds += 1
                while ready:
                    emit_add(ready.pop(0), done_adds)
                    done_adds += 1

                # ---- exp (undo RSCALE) + row sums, halves so the
                # transpose pipeline starts sooner ----
                Ebf = smp.tile([P, B], BF16, tag="Ebf")
                z2 = smp.tile([P, 2], F32, tag="z2")
                z = smp.tile([P, 1], F32, tag="z")
                rz = smp.tile([P, 1], F32, tag="rz")
                for jh in range(2):
                    nc.scalar.activation(
                        Ebf[:, jh * 512:(jh + 1) * 512],
                        attn_ps[:, jh * 512:(jh + 1) * 512], AF.Exp,
                        scale=1.0 / RSCALE, accum_out=z2[:, jh:jh + 1],
                    )
                nc.vector.tensor_tensor(
                    z[:, :], z2[:, 0:1], z2[:, 1:2], mybir.AluOpType.add
                )
                nc.vector.reciprocal(rz[:, :], z[:, :])

            # ---- transposes + output matmul (bf16) ----
            with (
                tc.tile_pool(name="tp", bufs=4, space="PSUM") as tp,
                tc.tile_pool(name="op", bufs=1, space="PSUM") as op,
            ):
                out_ps = op.tile([P, D], F32, tag="out")
                for jc in range(8):
                    ptile = tp.tile([P, P], BF16, tag="tp", name=f"tp{jc}")
                    nc.tensor.transpose(
                        ptile[:, :], Ebf[:, jc * P:(jc + 1) * P], ident[:, :]
                    )
                    et = etp.tile([P, P], BF16, tag="et", name=f"et{jc}")
                    nc.scalar.copy(et[:, :], ptile[:, :])
                    nc.tensor.matmul(
                        out_ps[:, :],
                        lhsT=et[:, :],
                        rhs=xc[jc],
                        start=(jc == 0),
                        stop=(jc == 7),
                    )
                out_sb = smp.tile([P, D], F32, tag="osb")
                nc.scalar.activation(
                    out_sb[:, :], out_ps[:, :], AF.Copy, scale=rz[:, :]
                )
                nc.sync.dma_start(out_d[:, :], out_sb[:, :])
    _split_excess_waits(nc)
    return nc


_NC_CACHE = None


def _get_nc():
    global _NC_CACHE
    if _NC_CACHE is None:
        _NC_CACHE = build_nc()
    return _NC_CACHE


def make_in_maps(x, q, R):
    x = np.asarray(x, dtype=np.float32)
    q = np.asarray(q)
    R = np.asarray(R, dtype=np.float32)
    bf = ml_dtypes.bfloat16
    f8 = ml_dtypes.float8_e4m3

    xT = np.ascontiguousarray(x.T)                        # [D, B]
    # xt_dr[p, i*B + j] = x[j, i*128+p], fp8
    xt_p = np.ascontiguousarray(
        xT.reshape(2, P, B).transpose(1, 0, 2).reshape(P, 2 * B)).astype(f8)
    # rt[p, i*41 + kk] = R[kk+1, i*128+p] * 512
    rts = (R.T[:, 1:] * RSCALE)                           # [D, 41]
    rt_p = np.ascontiguousarray(
        rts.reshape(2, P, NK).transpose(1, 0, 2).reshape(P, 2 * NK)).astype(bf)
    # xc[p, jc*D + d] = x[jc*128+p, d], bf16
    x_p = np.ascontiguousarray(
        x.reshape(8, P, D).transpose(1, 0, 2).reshape(P, 8 * D)).astype(bf)

    q32 = q.astype(np.int32)
    # dual identity for DoubleRow pair-adds: di[p, i*128+m] = (m == p)
    eye8 = np.eye(P, dtype=np.float32).astype(f8)
    di = np.concatenate([eye8, eye8], axis=1)              # [128, 256]
    in_maps = []
    for c in range(NCORES):
        rows = slice(c * P, (c + 1) * P)
        qb = q32[rows]                                     # [128, B]
        # one-hot lane masks, fp8-lane 0xFF packed little-endian into u16
        mbytes = np.zeros((P, NK, B), dtype=np.uint8)
        for k in range(1, NREL):
            mbytes[:, k - 1, :] = np.where(qb == k, 0xFF, 0)
        mbytes = np.concatenate(
            [mbytes, np.zeros((P, 1, B), dtype=np.uint8)], axis=1)
        mku = np.ascontiguousarray(
            mbytes.reshape(P, (NK + 1) * B)).view(np.uint16)
        xti = xT[:, rows]                                  # [D, 128]
        xti_p = np.ascontiguousarray(
            xti.reshape(2, P, P).transpose(1, 0, 2).reshape(P, 2 * P)
        ).astype(bf)
        in_maps.append(
            {"xt": xt_p, "xti": xti_p, "rt": rt_p, "mku": mku, "di": di,
             "xc": x_p}
        )
    return in_maps


def kernel(x, x_mask, q, f, R_emb):
    in_maps = make_in_maps(x, q, R_emb)
    res = run_bass_kernel_spmd(_get_nc(), in_maps, core_ids=list(range(NCORES)))
    out = np.concatenate([res.results[c]["out"] for c in range(NCORES)], axis=0)
    return out


# revision 15
# speedup vs baseline: 1.1326x; 1.0565x over previous
"""KnowledgeAwareAttention TRN2 kernel — flat masked-sum architecture.

attn[i,j] = sum_d R_emb[q[i,j],d] * x[j,d] * x[i,d]
out = softmax(attn, -1) @ x

Per core (128 rows):
  attn = sum_{k=1..41} 1[q==k] * T_k   (T_0 == 0: R row 0 is zeroed)
  - PE: T_k = (x_I*R_k*512) @ x^T via fp8e4 DoubleRow matmuls (256-contraction
    per instruction, ldweights pipelined away).
  - Planes are evacuated PSUM->SBUF as fp8 into pair-tiles (ScalarE, with a
    VectorE share for balance), then masked IN PLACE by a single
    tensor_tensor(bitwise_and) against host-packed one-hot lane masks
    (0xFF per selected fp8 lane, uint16-packed) - ~2x cheaper than any
    predicated/stt op on DVE.
  - PE accumulates masked pairs into the attn PSUM tile via dual-identity
    DoubleRow matmuls (two planes per 512-col call).
  - exp on ScalarE with scale=1/512 (undoes the fp8 range scale) + fused
    row-sum; reciprocal on VectorE; transposes + output matmul in bf16.
  - lh prep (x_I * R * 512 in fp8 DR layout) in ONE VectorE broadcast op.
"""

import numpy as np
import ml_dtypes

import concourse.bass as bass
import concourse.mybir as mybir
import concourse.tile as tile
from concourse.bass_utils import run_bass_kernel_spmd
from concourse.masks import make_identity

B = 1024
D = 256
NREL = 42
NK = NREL - 1  # planes 1..41; plane 0 is identically zero
NCORES = 8
P = 128
F32 = mybir.dt.float32
BF16 = mybir.dt.bfloat16
FP8 = mybir.dt.float8e4
AF = mybir.ActivationFunctionType
DRM = mybir.MatmulPerfMode.DoubleRow
RSCALE = 512.0  # fp8 range scale folded into R; undone in exp

# planes evacuated by VectorE instead of ScalarE (engine balance)
DVE_EVAC = set(range(3, 42, 4))  # 10 planes
# pair-adds lag the plane pipeline by this many pairs so the PE never
# stalls waiting for the evac+mask chain
ADD_LAG = 2


def _patch_tile_tail_drain():
    """This container's walrus rejects >1 sync-wait command on the
    kernel-tail SP Drain. Split the waits across SP nops."""
    import concourse.mybir as mybir_
    import concourse.tile as tile_

    def _drain_and_barrier(self, tick_clock, wait_clock):
        nc = self.nc
        drain_inst = nc.sync.drain()
        wait_clock.add_sem_waits(
            drain_inst.ins, tile_.ScopedClock({None: tick_clock.global_clock})
        )
        si = drain_inst.ins.sync_info
        waits = list(si.on_wait) if si and si.on_wait else []
        if len(waits) > 1:
            si.on_wait = waits[:1]
            for w in waits[1:]:
                nop = nc.sync.nop(nofuse=True)
                nop.ins.sync_info = mybir_.SyncInfo(on_wait=[w], on_update=[])
        nc.all_engine_barrier()
        assert self.sems is not None
        popped = nc._tile_sem_poison_stack.pop()
        assert popped is self._sem_poison
        nc.clear_and_free_semaphores(list(self.sems.allocated().values()))

    tile_.TileContext._drain_and_barrier = _drain_and_barrier


_patch_tile_tail_drain()


_MAX_WAITS = 1


def _split_excess_waits(nc: bass.Bass, max_waits: int = _MAX_WAITS) -> None:
    """This container's walrus caps the number of sync-wait commands one
    instruction may carry. Move excess waits onto same-engine NoOps."""
    cnt = 0
    for wrapper in nc.bb_map.values():
        bb = wrapper.bb
        old = list(bb.instructions)
        new = []
        changed = False
        for ins in old:
            si = ins.sync_info
            waits = list(si.on_wait) if si and si.on_wait else []
            if len(waits) > max_waits:
                changed = True
                si.on_wait = waits[:max_waits]
                rest = waits[max_waits:]
                for i in range(0, len(rest), max_waits):
                    nop = mybir.InstNoOp(name=f"waitnop{cnt}", ins=[], outs=[])
                    cnt += 1
                    nop.engine = ins.engine
                    nop.sync_info = mybir.SyncInfo(
                        on_wait=rest[i:i + max_waits], on_update=[]
                    )
                    new.append(nop)
            new.append(ins)
        if changed:
            bb.instructions = new


def build_nc() -> bass.Bass:
    nc = bass.Bass()
    xt_d = nc.dram_tensor("xt", [P, 2 * B], FP8, kind="ExternalInput")
    xti_d = nc.dram_tensor("xti", [P, 2 * P], BF16, kind="ExternalInput")
    rt_d = nc.dram_tensor("rt", [P, 2 * NK], BF16, kind="ExternalInput")
    mku_d = nc.dram_tensor("mku", [P, (NK + 1) * 512], mybir.dt.uint16,
                           kind="ExternalInput")
    di_d = nc.dram_tensor("di", [P, 2 * P], FP8, kind="ExternalInput")
    xc_d = nc.dram_tensor("xc", [P, 8 * D], BF16, kind="ExternalInput")
    out_d = nc.dram_tensor("out", [P, D], F32, kind="ExternalOutput")

    with tile.TileContext(nc) as tc:
        with (
            tc.tile_pool(name="const", bufs=1) as const,
            tc.tile_pool(name="lh", bufs=1) as lhp,
            tc.tile_pool(name="pb", bufs=3) as pbp,
            tc.tile_pool(name="mk", bufs=5) as mkp,
            tc.tile_pool(name="sm", bufs=1) as smp,
            tc.tile_pool(name="et", bufs=4) as etp,
        ):
            # ---- loads ----
            xt_t = const.tile([P, 2 * B], FP8, tag="xt", name="xt_t")
            xti_t = const.tile([P, 2 * P], BF16, tag="xti", name="xti_t")
            rt_t = const.tile([P, 2 * NK], BF16, tag="rt", name="rt_t")
            mku_t = const.tile([P, (NK + 1) * 512], mybir.dt.uint16, tag="mku",
                               name="mku_t")
            di_t = const.tile([P, 2 * P], FP8, tag="di", name="di_t")
            xc_t = const.tile([P, 8 * D], BF16, tag="xc", name="xc_t")
            nc.sync.dma_start(xti_t[:, :], xti_d[:, :])
            nc.sync.dma_start(rt_t[:, :], rt_d[:, :])
            nc.sync.dma_start(xt_t[:, :], xt_d[:, :])
            nc.sync.dma_start(di_t[:, :], di_d[:, :])
            # masks split into 3 chunks so early planes' masks arrive first
            nc.sync.dma_start(mku_t[:, :8 * 512], mku_d[:, :8 * 512])
            nc.sync.dma_start(mku_t[:, 8 * 512:24 * 512],
                              mku_d[:, 8 * 512:24 * 512])
            nc.sync.dma_start(mku_t[:, 24 * 512:], mku_d[:, 24 * 512:])
            nc.sync.dma_start(xc_t[:, :], xc_d[:, :])
            xc = [xc_t[:, j * D:(j + 1) * D] for j in range(8)]
            ident = const.tile([P, P], BF16, tag="ident")
            make_identity(nc, ident[:, :])

            # ---- prep: lh[p, kk, i, m] = xti[p, i*128+m] * rt[p, i*41+kk]
            # (kk = k-1; fp8 out, DoubleRow block layout [2, 128] per plane)
            lh_t = lhp.tile([P, NK * 2 * P], FP8, tag="lh", name="lh_t")

            def emit_prep(k0, nk):
                in0 = (
                    xti_t[:, :]
                    .rearrange("p (i m) -> p i m", i=2)
                    .unsqueeze(1)
                    .broadcast_to([P, nk, 2, P])
                )
                in1 = (
                    rt_t[:, :]
                    .rearrange("p (i k) -> p i k", i=2)[:, :, k0:k0 + nk]
                    .transpose([0, 2, 1])
                    .unsqueeze(3)
                    .broadcast_to([P, nk, 2, P])
                )
                outv = lh_t[
                    :, k0 * 2 * P:(k0 + nk) * 2 * P
                ].rearrange("p (k i m) -> p k i m", k=nk, i=2)
                nc.vector.tensor_tensor(outv, in0, in1, mybir.AluOpType.mult)

            emit_prep(0, 6)
            emit_prep(6, 8)
            # chunks for planes 15..41 are emitted inside the plane loop so
            # VectorE's evac/mask ops are not queued behind them
            lh = [
                lh_t[:, kk * 2 * P:(kk + 1) * 2 * P].rearrange(
                    "p (i m) -> p i m", i=2
                )
                for kk in range(NK)
            ]
            xt_dr = xt_t[:, :].rearrange("p (i j) -> p i j", i=2)

            # ---- planes + masked accumulation ----
            di_dr = di_t[:, :].rearrange("p (i m) -> p i m", i=2)
            NPAIRS = (NK + 1) // 2  # 21; pair 20 has a zeroed second half
            with (
                tc.tile_pool(name="pp", bufs=3, space="PSUM") as pp,
                tc.tile_pool(name="ap", bufs=1, space="PSUM") as app,
            ):
                attn_ps = app.tile([P, B], F32, tag="attn")
                ready = []  # completed masked pair tiles awaiting their add
                done_adds = 0

                def emit_add(pair_t, idx):
                    pr = pair_t[:, :].rearrange("p (i j) -> p i j", i=2)
                    for jh in range(2):
                        nc.tensor.matmul(
                            attn_ps[:, jh * 512:(jh + 1) * 512],
                            lhsT=di_dr,
                            rhs=pr[:, :, jh * 512:(jh + 1) * 512],
                            start=(idx == 0),
                            stop=(idx == NPAIRS - 1),
                            perf_mode=DRM,
                        )

                pair = None
                for k in range(1, NREL):
                    kk = k - 1
                    t, half = kk // 2, kk % 2
                    pt = pp.tile([P, B], F32, tag="plane", name=f"t{k}")
                    for jh in range(2):
                        nc.tensor.matmul(
                            pt[:, jh * 512:(jh + 1) * 512],
                            lhsT=lh[kk],
                            rhs=xt_dr[:, :, jh * 512:(jh + 1) * 512],
                            start=True,
                            stop=True,
                            perf_mode=DRM,
                        )
                    if half == 0:
                        pair = mkp.tile([P, 2 * B], FP8, tag="mk",
                                        name=f"mk{t}")
                    slot = pair[:, half * B:(half + 1) * B]
                    if k in DVE_EVAC:
                        nc.vector.tensor_copy(slot, pt[:, :])
                    else:
                        nc.scalar.copy(slot, pt[:, :])
                    if half == 1 or t == NPAIRS - 1:
                        # one pair-wide in-place AND; mask slot NK zeroes the
                        # garbage second half of the last (odd) pair
                        pair16 = pair[:, :].bitcast(mybir.dt.uint16)
                        nc.vector.tensor_tensor(
                            pair16,
                            pair16,
                            mku_t[:, (2 * t) * 512:(2 * t + 2) * 512],
                            mybir.AluOpType.bitwise_and,
                        )
                        ready.append(pair)
                    if k == 4:
                        emit_prep(14, 7)
                    elif k == 8:
                        emit_prep(21, 7)
                    elif k == 12:
                        emit_prep(28, 7)
                    elif k == 16:
                        emit_prep(35, NK - 35)
                    while len(ready) > ADD_LAG:
                        emit_add(ready.pop(0), done_adds)
                        done_adds += 1
                while ready:
                    emit_add(ready.pop(0), done_adds)
                    done_adds += 1

                # ---- exp (undo RSCALE) + row sums, halves so the
                # transpose pipeline starts sooner ----
                Ebf = smp.tile([P, B], BF16, tag="Ebf")
                z2 = smp.tile([P, 2], F32, tag="z2")
                z = smp.tile([P, 1], F32, tag="z")
                rz = smp.tile([P, 1], F32, tag="rz")
                for jh in range(2):
                    nc.scalar.activation(
                        Ebf[:, jh * 512:(jh + 1) * 512],
                        attn_ps[:, jh * 512:(jh + 1) * 512], AF.Exp,
                        scale=1.0 / RSCALE, accum_out=z2[:, jh:jh + 1],
                    )
                nc.vector.tensor_tensor(
                    z[:, :], z2[:, 0:1], z2[:, 1:2], mybir.AluOpType.add
                )
                nc.vector.reciprocal(rz[:, :], z[:, :])

            # ---- transposes + output matmul (bf16) ----
            with (
                tc.tile_pool(name="tp", bufs=4, space="PSUM") as tp,
                tc.tile_pool(name="op", bufs=1, space="PSUM") as op,
            ):
                out_ps = op.tile([P, D], F32, tag="out")
                for jc in range(8):
                    ptile = tp.tile([P, P], BF16, tag="tp", name=f"tp{jc}")
                    nc.tensor.transpose(
                        ptile[:, :], Ebf[:, jc * P:(jc + 1) * P], ident[:, :]
                    )
                    et = etp.tile([P, P], BF16, tag="et", name=f"et{jc}")
                    if jc % 2 == 0:
                        nc.scalar.copy(et[:, :], ptile[:, :])
                    else:
                        nc.vector.tensor_copy(et[:, :], ptile[:, :])
                    nc.tensor.matmul(
                        out_ps[:, :],
                        lhsT=et[:, :],
                        rhs=xc[jc],
                        start=(jc == 0),
                        stop=(jc == 7),
                    )
                out_sb = smp.tile([P, D], F32, tag="osb")
                nc.scalar.activation(
                    out_sb[:, :], out_ps[:, :], AF.Copy, scale=rz[:, :]
                )
                nc.sync.dma_start(out_d[:, :], out_sb[:, :])
    _split_excess_waits(nc)
    return nc


_NC_CACHE = None


def _get_nc():
    global _NC_CACHE
    if _NC_CACHE is None:
        _NC_CACHE = build_nc()
    return _NC_CACHE


def make_in_maps(x, q, R):
    x = np.asarray(x, dtype=np.float32)
    q = np.asarray(q)
    R = np.asarray(R, dtype=np.float32)
    bf = ml_dtypes.bfloat16
    f8 = ml_dtypes.float8_e4m3

    xT = np.ascontiguousarray(x.T)                        # [D, B]
    # xt_dr[p, i*B + j] = x[j, i*128+p], fp8
    xt_p = np.ascontiguousarray(
        xT.reshape(2, P, B).transpose(1, 0, 2).reshape(P, 2 * B)).astype(f8)
    # rt[p, i*41 + kk] = R[kk+1, i*128+p] * 512
    rts = (R.T[:, 1:] * RSCALE)                           # [D, 41]
    rt_p = np.ascontiguousarray(
        rts.reshape(2, P, NK).transpose(1, 0, 2).reshape(P, 2 * NK)).astype(bf)
    # xc[p, jc*D + d] = x[jc*128+p, d], bf16
    x_p = np.ascontiguousarray(
        x.reshape(8, P, D).transpose(1, 0, 2).reshape(P, 8 * D)).astype(bf)

    q32 = q.astype(np.int32)
    # dual identity for DoubleRow pair-adds: di[p, i*128+m] = (m == p)
    eye8 = np.eye(P, dtype=np.float32).astype(f8)
    di = np.concatenate([eye8, eye8], axis=1)              # [128, 256]
    in_maps = []
    for c in range(NCORES):
        rows = slice(c * P, (c + 1) * P)
        qb = q32[rows]                                     # [128, B]
        # one-hot lane masks, fp8-lane 0xFF packed little-endian into u16
        mbytes = np.zeros((P, NK, B), dtype=np.uint8)
        for k in range(1, NREL):
            mbytes[:, k - 1, :] = np.where(qb == k, 0xFF, 0)
        mbytes = np.concatenate(
            [mbytes, np.zeros((P, 1, B), dtype=np.uint8)], axis=1)
        mku = np.ascontiguousarray(
            mbytes.reshape(P, (NK + 1) * B)).view(np.uint16)
        xti = xT[:, rows]                                  # [D, 128]
        xti_p = np.ascontiguousarray(
            xti.reshape(2, P, P).transpose(1, 0, 2).reshape(P, 2 * P)
        ).astype(bf)
        in_maps.append(
            {"xt": xt_p, "xti": xti_p, "rt": rt_p, "mku": mku, "di": di,
             "xc": x_p}
        )
    return in_maps


def kernel(x, x_mask, q, f, R_emb):
    in_maps = make_in_maps(x, q, R_emb)
    res = run_bass_kernel_spmd(_get_nc(), in_maps, core_ids=list(range(NCORES)))
    out = np.concatenate([res.results[c]["out"] for c in range(NCORES)], axis=0)
    return out
